# revision 27
# baseline (speedup 1.0000x reference)
"""Trainium2 Bass kernel for nn_DuelingDQN (2-layer LSTM + dueling-advantage MLP).

Strategy
--------
Data-parallel over batch: B=4096 is split as 512 per NeuronCore across 8 cores;
weights are replicated. On each core everything is kept in a transposed layout
(features on the SBUF partition dim, batch on the free dim), so the sequential
T=100 recurrence runs as a chain of bf16 matmuls (fp32 PSUM accumulation):

    gates.T (1024 x 512) = W.T-slices.T @ [x_t.T ; h.T]

256-row feature tensors (h, c, per-gate activations) are stored "folded" as
(128, 2*512) SBUF tiles — free-dim halves are feature rows [0:128) / [128:256) —
which halves the elementwise-op count. The input x is pre-transposed and cast
to bf16 on the host (numpy), so the device never transposes anything.

Per step: 64 LSTM matmuls + 6 MLP matmuls on PE, gate sigmoid/tanh (+bias) on
ACT straight out of PSUM, cell updates on DVE (c stays fp32, h is written as
bf16 for the next matmul). The MLP head for step t is emitted in the middle of
step t+1 so PE never stalls waiting for h1's ACT/DVE tail.

The walrus build in this container encodes at most ONE sync-wait per
instruction; Tile emits several. `_split_multiwaits` post-processes the BIR
JSON, hoisting extra waits onto injected same-engine EventSemaphore
instructions immediately before the owner (engine streams are in-order, so
this is semantically identical).
"""

import json
import sys
import types
from contextlib import ExitStack

import numpy as np

sys.path.insert(0, "/opt/trn_rl_repo")

import ml_dtypes  # noqa: E402

N_CORES = 8
B, T, IN, H = 4096, 100, 140, 256
BC = B // N_CORES  # 512 batch per core
G = 4 * H  # 1024 gate rows
BF16 = ml_dtypes.bfloat16


# --------------------------------------------------------------------------
# BIR post-processing: split multi-wait instructions (see module docstring)
# --------------------------------------------------------------------------
def _split_multiwaits(bir: dict) -> int:
    ctr = 0
    for f in bir["functions"]:
        for blk in f["blocks"]:
            new_insts = []
            for ins in blk["instructions"]:
                si = ins.get("sync_info")
                waits = (si or {}).get("on_wait") or []
                if len(waits) > 1:
                    for w in waits[:-1]:
                        ctr += 1
                        new_insts.append(
                            {
                                "debug": ins.get("debug", 0),
                                "engine": ins["engine"],
                                "ins": [],
                                "outs": [],
                                "name": f"antsplitw-{ctr}",
                                "opcode": "EventSemaphore",
                                "sync_info": {"on_update": [], "on_wait": [w]},
                            }
                        )
                    si["on_wait"] = [waits[-1]]
                new_insts.append(ins)
            blk["instructions"] = new_insts
    return ctr


def _patch_bass(nc):
    import concourse.mybir as mybir

    def to_json_bytes(self):
        j = json.loads(mybir.module_to_json_bytes(self.m))
        _split_multiwaits(j)
        return json.dumps(j).encode()

    nc.to_json_bytes = types.MethodType(to_json_bytes, nc)
    return nc


# --------------------------------------------------------------------------
# Module build
# --------------------------------------------------------------------------
def build_module(b_a2_val: float, T_steps: int = T):
    import concourse.bass as bass
    import concourse.tile as tile
    from concourse import mybir

    f32 = mybir.dt.float32
    bf16 = mybir.dt.bfloat16
    AF = mybir.ActivationFunctionType
    ALU = mybir.AluOpType

    nc = bass.Bass("TRN2", target_bir_lowering=False, debug=False)

    # x is extended with a constant ones-row (index IN) so the layer-0 bias
    # rides in the tail matmul (wih0 row IN = b0) — frees ACT from per-half
    # bias and lets layer-0 gates use single folded 1024-wide ACT ops.
    xT_d = nc.dram_tensor("xT", (T_steps, IN + 1, BC), bf16, kind="ExternalInput").ap()
    wih0_d = nc.dram_tensor("wih0", (128, G), bf16, kind="ExternalInput").ap()
    # x-tail weights (12 rows of W_ih0.T + b0 row), replicated at partition
    # offsets 0/32/64/96 (host-prepared)
    wih0b_d = nc.dram_tensor("wih0b", (96 + (IN + 1 - 128), G), bf16,
                             kind="ExternalInput").ap()
    whh0_d = nc.dram_tensor("whh0", (128, 2 * G), bf16, kind="ExternalInput").ap()
    wih1_d = nc.dram_tensor("wih1", (128, 2 * G), bf16, kind="ExternalInput").ap()
    whh1_d = nc.dram_tensor("whh1", (128, 2 * G), bf16, kind="ExternalInput").ap()
    wa1_d = nc.dram_tensor("wa1", (128, 2 * H), bf16, kind="ExternalInput").ap()
    wa2_d = nc.dram_tensor("wa2", (128, 2), bf16, kind="ExternalInput").ap()
    bias1_d = nc.dram_tensor("bias1", (128, 8), f32, kind="ExternalInput").ap()
    ba1_d = nc.dram_tensor("ba1", (128, 2), f32, kind="ExternalInput").ap()
    o_d = nc.dram_tensor("o", (T_steps, BC), f32, kind="ExternalOutput").ap()

    GATE_FUNCS = [AF.Sigmoid, AF.Sigmoid, AF.Tanh, AF.Sigmoid]  # i, f, g, o

    with tile.TileContext(nc) as tc, ExitStack() as ctx:
        persist = ctx.enter_context(tc.tile_pool(name="persist", bufs=1))
        xpool = ctx.enter_context(tc.tile_pool(name="xpool", bufs=4))
        gpool = ctx.enter_context(tc.tile_pool(name="gates_sb", bufs=2))
        tpool = ctx.enter_context(tc.tile_pool(name="tmp_sb", bufs=2))
        psg = ctx.enter_context(tc.tile_pool(name="ps_gates", bufs=3, space="PSUM"))
        pso = ctx.enter_context(tc.tile_pool(name="ps_out", bufs=2, space="PSUM"))

        def load(name, dram_ap, shape, dt):
            t = persist.tile(shape, dt, tag=name, name=name)
            nc.sync.dma_start(t[:], dram_ap)
            return t

        KT = IN + 1 - 128  # 13 tail rows (12 x rows + ones row)
        wih0a = load("wih0a", wih0_d[:], [128, G], bf16)
        # tail weights replicated at partition offsets 0/32/64/96 so four K=13
        # tail matmuls can run concurrently in distinct PE row-groups
        wih0b = load("wih0b", wih0b_d[:], [96 + KT, G], bf16)
        whh0 = load("whh0", whh0_d[:], [128, 2 * G], bf16)
        wih1 = load("wih1", wih1_d[:], [128, 2 * G], bf16)
        whh1 = load("whh1", whh1_d[:], [128, 2 * G], bf16)
        wa1 = load("wa1", wa1_d[:], [128, 2 * H], bf16)
        wa2 = load("wa2", wa2_d[:], [128, 2], bf16)
        bias1 = load("bias1", bias1_d[:], [128, 8], f32)
        ba1 = load("ba1", ba1_d[:], [128, 2], f32)

        h0 = persist.tile([128, 2 * BC], bf16, tag="h0", name="h0")
        h1 = persist.tile([128, 2 * BC], bf16, tag="h1", name="h1")
        c0 = persist.tile([128, 2 * BC], f32, tag="c0", name="c0")
        c1 = persist.tile([128, 2 * BC], f32, tag="c1", name="c1")

        def make_tiles(t, lname, g):
            ps = psg.tile([128, 2 * BC], f32, tag="gates", name=f"ps_{lname}{g}_{t}")
            sb = gpool.tile([128, 2 * BC], bf16, tag=f"g{g}",
                            name=f"sb_{lname}{g}_{t}")
            return ps, sb

        def cell(t, lname, gates, h, c):
            gi, gf, gg, go = gates
            if t > 0:
                # c*f first: it only waits on the f-gate ACT (ready mid-layer)
                nc.vector.tensor_mul(c[:], c[:], gf[:])
                t1 = tpool.tile([128, 2 * BC], bf16, tag="t1", name=f"t1_{lname}_{t}")
                nc.vector.tensor_mul(t1[:], gi[:], gg[:])
                nc.vector.tensor_add(c[:], c[:], t1[:])
            else:
                nc.vector.tensor_mul(c[:], gi[:], gg[:])
            # halves: the consumer's first K-tile matmul only needs h[:, 0:BC]
            tc_t = tpool.tile([128, 2 * BC], bf16, tag="tanhc", name=f"tc_{lname}_{t}")
            for j in range(2):
                sl = slice(j * BC, (j + 1) * BC)
                nc.scalar.activation(tc_t[:, sl], c[:, sl], AF.Tanh)
                nc.vector.tensor_mul(h[:, sl], go[:, sl], tc_t[:, sl])

        def l0_layer(t, xa, xb4):
            """Layer 0 for step t: bias rides in the K=13 tail matmul whose
            four per-quad instances issue back-to-back at distinct
            tile_positions (row-groups 0/32/64/96) and run concurrently."""
            gates = [None] * 4
            for q in (0, 1):
                pair = (2 * q, 2 * q + 1)
                tiles = {}
                for g in pair:
                    tiles[g] = make_tiles(t, "l0", g)
                    gates[g] = tiles[g][1]

                def quad(which):
                    for idx in range(4):
                        m = 4 * q + idx
                        g, j = divmod(m, 2)
                        out = tiles[g][0][:, j * BC : (j + 1) * BC]
                        col = 128 * m
                        if which == "xb":
                            nc.tensor.matmul(
                                out,
                                wih0b[32 * idx : 32 * idx + KT, col : col + 128],
                                xb4[32 * idx : 32 * idx + KT, :],
                                start=True, stop=False,
                                tile_position=(32 * idx, 0),
                            )
                        elif which == "xa":
                            nc.tensor.matmul(out, wih0a[:, col : col + 128], xa[:],
                                             start=False, stop=(t == 0))
                        else:
                            for k in range(2):
                                nc.tensor.matmul(
                                    out,
                                    whh0[:, k * G + col : k * G + col + 128],
                                    h0[:, k * BC : (k + 1) * BC],
                                    start=False, stop=(k == 1),
                                )

                quad("xb")
                quad("xa")
                if t > 0:
                    quad("hh")
                for g in pair:
                    ps, sb = tiles[g]
                    # bias already in PSUM via the ones-row: folded 1024-wide ACT
                    nc.scalar.activation(sb[:], ps[:], GATE_FUNCS[g])
            cell(t, "l0", gates, h0, c0)

        def l1_layer(t):
            """Layer 1 for step t; per gate-pair all hh matmuls precede ih
            matmuls to cover the h0[t] ACT/DVE tail with PE work."""
            gates = [None] * 4
            for g0 in (0, 2):
                tiles = {g: make_tiles(t, "l1", g) for g in (g0, g0 + 1)}
                for g in (g0, g0 + 1):
                    gates[g] = tiles[g][1]
                rhs_ih = [h0[:, 0:BC], h0[:, BC : 2 * BC]]
                if t > 0:
                    for g in (g0, g0 + 1):
                        for j in range(2):
                            col = 128 * (2 * g + j)
                            out = tiles[g][0][:, j * BC : (j + 1) * BC]
                            for k in range(2):
                                nc.tensor.matmul(
                                    out,
                                    whh1[:, k * G + col : k * G + col + 128],
                                    h1[:, k * BC : (k + 1) * BC],
                                    start=(k == 0), stop=False,
                                )
                for g in (g0, g0 + 1):
                    ps, sb = tiles[g]
                    for j in range(2):
                        m = 2 * g + j
                        col = 128 * m
                        out = ps[:, j * BC : (j + 1) * BC]
                        for k in range(2):
                            nc.tensor.matmul(
                                out,
                                wih1[:, k * G + col : k * G + col + 128],
                                rhs_ih[k],
                                start=(t == 0 and k == 0), stop=(k == 1),
                            )
                        nc.scalar.activation(sb[:, j * BC : (j + 1) * BC], out,
                                             GATE_FUNCS[g], bias=bias1[:, m : m + 1])
            cell(t, "l1", gates, h1, c1)

        def mlp_head(t):
            """Advantage head for step t; reads current h1 contents."""
            ps_a = psg.tile([128, 2 * BC], f32, tag="gates", name=f"ps_a1_{t}")
            relu = tpool.tile([128, 2 * BC], bf16, tag="relu", name=f"relu_{t}")
            for j in range(2):
                out = ps_a[:, j * BC : (j + 1) * BC]
                for k in range(2):
                    nc.tensor.matmul(
                        out,
                        wa1[:, k * H + 128 * j : k * H + 128 * j + 128],
                        h1[:, k * BC : (k + 1) * BC],
                        start=(k == 0), stop=(k == 1),
                    )
                nc.scalar.activation(relu[:, j * BC : (j + 1) * BC], out,
                                     AF.Relu, bias=ba1[:, j : j + 1])
            ps_o = pso.tile([1, BC], f32, tag="po", name=f"ps_o_{t}")
            for k in range(2):
                nc.tensor.matmul(ps_o[:], wa2[:, k : k + 1],
                                 relu[:, k * BC : (k + 1) * BC],
                                 start=(k == 0), stop=(k == 1))
            # b_a2 is added on the host; ACT drains PSUM (ACT has slack)
            osb = tpool.tile([1, BC], f32, tag="osb", name=f"osb_{t}")
            nc.scalar.copy(osb[:], ps_o[:])
            nc.sync.dma_start(o_d[t : t + 1, :], osb[:])

        for t in range(T_steps):
            xa = xpool.tile([128, BC], bf16, tag="xa", name=f"xa_{t}")
            nc.sync.dma_start(xa[:], xT_d[t, 0:128, :])
            xb4 = xpool.tile([96 + KT, BC], bf16, tag="xb", name=f"xb_{t}")
            for i in range(4):
                nc.sync.dma_start(xb4[32 * i : 32 * i + KT, :],
                                  xT_d[t, 128 : IN + 1, :])

            l0_layer(t, xa, xb4)
            if t > 0:
                mlp_head(t - 1)
            l1_layer(t)
        mlp_head(T_steps - 1)

    return _patch_bass(nc)


# --------------------------------------------------------------------------
# Host-side input prep / output assembly
# --------------------------------------------------------------------------
def _fold(wT: np.ndarray) -> np.ndarray:
    """(2K, M) -> (128, 2M): free halves are K-rows [0:128) / [128:256)."""
    k2, m = wT.shape
    assert k2 == 256
    return np.ascontiguousarray(
        wT.reshape(2, 128, m).transpose(1, 0, 2).reshape(128, 2 * m)
    )


def prepare_in_maps(inputs: dict) -> list[dict]:
    f32 = np.float32
    W_ih0 = np.asarray(inputs["W_ih0"], f32)
    W_hh0 = np.asarray(inputs["W_hh0"], f32)
    W_ih1 = np.asarray(inputs["W_ih1"], f32)
    W_hh1 = np.asarray(inputs["W_hh1"], f32)
    W_a1 = np.asarray(inputs["W_a1"], f32)
    W_a2 = np.asarray(inputs["W_a2"], f32)

    b0 = np.asarray(inputs["b_ih0"], f32) + np.asarray(inputs["b_hh0"], f32)
    # x-tail weight block: rows 128:140 of W_ih0.T plus the b0 bias row
    # (multiplied by the ones-row appended to x), replicated at partition
    # offsets 0/32/64/96 for tile_position-packed K=13 matmuls
    KT = IN + 1 - 128
    tail = np.concatenate([W_ih0.T[128:IN], b0[None, :]], axis=0)  # (13, G)
    wih0b = np.zeros((96 + KT, tail.shape[1]), f32)
    for i in range(4):
        wih0b[32 * i : 32 * i + KT] = tail

    shared = {
        "wih0": np.ascontiguousarray(W_ih0.T[0:128]).astype(BF16),
        "wih0b": wih0b.astype(BF16),
        "whh0": _fold(W_hh0.T).astype(BF16),
        "wih1": _fold(W_ih1.T).astype(BF16),
        "whh1": _fold(W_hh1.T).astype(BF16),
        "wa1": _fold(W_a1.T).astype(BF16),
        "wa2": _fold(W_a2.T).astype(BF16),
        "bias1": np.ascontiguousarray(
            (np.asarray(inputs["b_ih1"], f32) + np.asarray(inputs["b_hh1"], f32))
            .reshape(8, 128).T),
        "ba1": np.ascontiguousarray(np.asarray(inputs["b_a1"], f32).reshape(2, 128).T),
    }

    x = np.asarray(inputs["x"], f32)  # (B, T, IN)
    t_steps = x.shape[1]
    xT = x.transpose(1, 2, 0)  # (T, IN, B) view
    in_maps = []
    for c in range(N_CORES):
        xc = np.empty((t_steps, IN + 1, BC), BF16)
        xc[:, :IN, :] = xT[:, :, c * BC : (c + 1) * BC].astype(BF16)
        xc[:, IN, :] = np.ones((), BF16)
        in_maps.append({"xT": xc, **shared})
    return in_maps


def assemble_output(results: list[dict], b_a2_val: float) -> np.ndarray:
    out_tb = np.concatenate([r["o"] for r in results], axis=1)  # (T, B)
    out_tb = out_tb + np.float32(b_a2_val)
    t_steps = out_tb.shape[0]
    return np.ascontiguousarray(out_tb.reshape(B, t_steps))


_module_cache: dict = {}


def get_module(b_a2_val: float):
    key = round(float(b_a2_val), 12)
    if key not in _module_cache:
        _module_cache[key] = build_module(float(b_a2_val))
    return _module_cache[key]


def kernel(**inputs) -> np.ndarray:
    from concourse import bass_utils

    b_a2_val = float(np.asarray(inputs["b_a2"], np.float32).reshape(-1)[0])
    nc = get_module(b_a2_val)
    in_maps = prepare_in_maps(inputs)
    res = bass_utils.run_bass_kernel_spmd(nc, in_maps, core_ids=list(range(N_CORES)))
    return assemble_output(res.results, b_a2_val)


# revision 29
# speedup vs baseline: 1.0012x; 1.0012x over previous
"""Trainium2 Bass kernel for nn_DuelingDQN (2-layer LSTM + dueling-advantage MLP).

Strategy
--------
Data-parallel over batch: B=4096 is split as 512 per NeuronCore across 8 cores;
weights are replicated. On each core everything is kept in a transposed layout
(features on the SBUF partition dim, batch on the free dim), so the sequential
T=100 recurrence runs as a chain of bf16 matmuls (fp32 PSUM accumulation):

    gates.T (1024 x 512) = W.T-slices.T @ [x_t.T ; h.T]

256-row feature tensors (h, c, per-gate activations) are stored "folded" as
(128, 2*512) SBUF tiles — free-dim halves are feature rows [0:128) / [128:256) —
which halves the elementwise-op count. The input x is pre-transposed and cast
to bf16 on the host (numpy), so the device never transposes anything.

Per step: 64 LSTM matmuls + 6 MLP matmuls on PE, gate sigmoid/tanh (+bias) on
ACT straight out of PSUM, cell updates on DVE (c stays fp32, h is written as
bf16 for the next matmul). The MLP head for step t is emitted in the middle of
step t+1 so PE never stalls waiting for h1's ACT/DVE tail.

The walrus build in this container encodes at most ONE sync-wait per
instruction; Tile emits several. `_split_multiwaits` post-processes the BIR
JSON, hoisting extra waits onto injected same-engine EventSemaphore
instructions immediately before the owner (engine streams are in-order, so
this is semantically identical).
"""

import json
import sys
import types
from contextlib import ExitStack

import numpy as np

sys.path.insert(0, "/opt/trn_rl_repo")

import ml_dtypes  # noqa: E402

N_CORES = 8
B, T, IN, H = 4096, 100, 140, 256
BC = B // N_CORES  # 512 batch per core
G = 4 * H  # 1024 gate rows
BF16 = ml_dtypes.bfloat16


# --------------------------------------------------------------------------
# BIR post-processing: split multi-wait instructions (see module docstring)
# --------------------------------------------------------------------------
def _split_multiwaits(bir: dict) -> int:
    ctr = 0
    for f in bir["functions"]:
        for blk in f["blocks"]:
            new_insts = []
            for ins in blk["instructions"]:
                si = ins.get("sync_info")
                waits = (si or {}).get("on_wait") or []
                if len(waits) > 1:
                    for w in waits[:-1]:
                        ctr += 1
                        new_insts.append(
                            {
                                "debug": ins.get("debug", 0),
                                "engine": ins["engine"],
                                "ins": [],
                                "outs": [],
                                "name": f"antsplitw-{ctr}",
                                "opcode": "EventSemaphore",
                                "sync_info": {"on_update": [], "on_wait": [w]},
                            }
                        )
                    si["on_wait"] = [waits[-1]]
                new_insts.append(ins)
            blk["instructions"] = new_insts
    return ctr


def _patch_bass(nc):
    import concourse.mybir as mybir

    def to_json_bytes(self):
        j = json.loads(mybir.module_to_json_bytes(self.m))
        _split_multiwaits(j)
        return json.dumps(j).encode()

    nc.to_json_bytes = types.MethodType(to_json_bytes, nc)
    return nc


# --------------------------------------------------------------------------
# Module build
# --------------------------------------------------------------------------
def build_module(b_a2_val: float, T_steps: int = T, opts: dict | None = None):
    opts = opts or {}
    import concourse.bass as bass
    import concourse.tile as tile
    from concourse import mybir

    f32 = mybir.dt.float32
    bf16 = mybir.dt.bfloat16
    AF = mybir.ActivationFunctionType
    ALU = mybir.AluOpType

    nc = bass.Bass("TRN2", target_bir_lowering=False, debug=False)

    # x is extended with a constant ones-row (index IN) so the layer-0 bias
    # rides in the tail matmul (wih0 row IN = b0) — frees ACT from per-half
    # bias and lets layer-0 gates use single folded 1024-wide ACT ops.
    xT_d = nc.dram_tensor("xT", (T_steps, IN + 1, BC), bf16, kind="ExternalInput").ap()
    wih0_d = nc.dram_tensor("wih0", (128, G), bf16, kind="ExternalInput").ap()
    # x-tail weights (12 rows of W_ih0.T + b0 row), replicated at partition
    # offsets 0/32/64/96 (host-prepared)
    wih0b_d = nc.dram_tensor("wih0b", (96 + (IN + 1 - 128), G), bf16,
                             kind="ExternalInput").ap()
    whh0_d = nc.dram_tensor("whh0", (128, 2 * G), bf16, kind="ExternalInput").ap()
    wih1_d = nc.dram_tensor("wih1", (128, 2 * G), bf16, kind="ExternalInput").ap()
    whh1_d = nc.dram_tensor("whh1", (128, 2 * G), bf16, kind="ExternalInput").ap()
    wa1_d = nc.dram_tensor("wa1", (128, 2 * H), bf16, kind="ExternalInput").ap()
    wa2_d = nc.dram_tensor("wa2", (128, 2), bf16, kind="ExternalInput").ap()
    bias1_d = nc.dram_tensor("bias1", (128, 8), f32, kind="ExternalInput").ap()
    ba1_d = nc.dram_tensor("ba1", (128, 2), f32, kind="ExternalInput").ap()
    o_d = nc.dram_tensor("o", (T_steps, BC), f32, kind="ExternalOutput").ap()

    GATE_FUNCS = [AF.Sigmoid, AF.Sigmoid, AF.Tanh, AF.Sigmoid]  # i, f, g, o

    with tile.TileContext(nc) as tc, ExitStack() as ctx:
        persist = ctx.enter_context(tc.tile_pool(name="persist", bufs=1))
        xpool = ctx.enter_context(tc.tile_pool(name="xpool", bufs=4))
        gpool = ctx.enter_context(tc.tile_pool(name="gates_sb", bufs=opts.get("gbufs", 3)))
        tpool = ctx.enter_context(tc.tile_pool(name="tmp_sb", bufs=opts.get("tbufs", 3)))
        psg = ctx.enter_context(tc.tile_pool(name="ps_gates", bufs=3, space="PSUM"))
        pso = ctx.enter_context(tc.tile_pool(name="ps_out", bufs=2, space="PSUM"))

        def load(name, dram_ap, shape, dt):
            t = persist.tile(shape, dt, tag=name, name=name)
            nc.sync.dma_start(t[:], dram_ap)
            return t

        KT = IN + 1 - 128  # 13 tail rows (12 x rows + ones row)
        wih0a = load("wih0a", wih0_d[:], [128, G], bf16)
        # tail weights replicated at partition offsets 0/32/64/96 so four K=13
        # tail matmuls can run concurrently in distinct PE row-groups
        wih0b = load("wih0b", wih0b_d[:], [96 + KT, G], bf16)
        whh0 = load("whh0", whh0_d[:], [128, 2 * G], bf16)
        wih1 = load("wih1", wih1_d[:], [128, 2 * G], bf16)
        whh1 = load("whh1", whh1_d[:], [128, 2 * G], bf16)
        wa1 = load("wa1", wa1_d[:], [128, 2 * H], bf16)
        wa2 = load("wa2", wa2_d[:], [128, 2], bf16)
        bias1 = load("bias1", bias1_d[:], [128, 8], f32)
        ba1 = load("ba1", ba1_d[:], [128, 2], f32)

        h0 = persist.tile([128, 2 * BC], bf16, tag="h0", name="h0")
        h1 = persist.tile([128, 2 * BC], bf16, tag="h1", name="h1")
        c0 = persist.tile([128, 2 * BC], f32, tag="c0", name="c0")
        c1 = persist.tile([128, 2 * BC], f32, tag="c1", name="c1")

        def make_tiles(t, lname, g):
            ps = psg.tile([128, 2 * BC], f32, tag="gates", name=f"ps_{lname}{g}_{t}")
            sb = gpool.tile([128, 2 * BC], bf16, tag=f"g{g}",
                            name=f"sb_{lname}{g}_{t}")
            return ps, sb

        def cell(t, lname, gates, h, c):
            gi, gf, gg, go = gates
            if t > 0:
                # c*f first: it only waits on the f-gate ACT (ready mid-layer)
                nc.vector.tensor_mul(c[:], c[:], gf[:])
                t1 = tpool.tile([128, 2 * BC], bf16, tag="t1", name=f"t1_{lname}_{t}")
                nc.vector.tensor_mul(t1[:], gi[:], gg[:])
                nc.vector.tensor_add(c[:], c[:], t1[:])
            else:
                nc.vector.tensor_mul(c[:], gi[:], gg[:])
            # halves: the consumer's first K-tile matmul only needs h[:, 0:BC]
            tc_t = tpool.tile([128, 2 * BC], bf16, tag="tanhc", name=f"tc_{lname}_{t}")
            for j in range(2):
                sl = slice(j * BC, (j + 1) * BC)
                nc.scalar.activation(tc_t[:, sl], c[:, sl], AF.Tanh)
                nc.vector.tensor_mul(h[:, sl], go[:, sl], tc_t[:, sl])

        def l0_layer(t, xa, xb4):
            """Layer 0 for step t: bias rides in the K=13 tail matmul whose
            four per-quad instances issue back-to-back at distinct
            tile_positions (row-groups 0/32/64/96) and run concurrently."""
            gates = [None] * 4
            for q in (0, 1):
                pair = (2 * q, 2 * q + 1)
                tiles = {}
                for g in pair:
                    tiles[g] = make_tiles(t, "l0", g)
                    gates[g] = tiles[g][1]

                def quad(which):
                    for idx in range(4):
                        m = 4 * q + idx
                        g, j = divmod(m, 2)
                        out = tiles[g][0][:, j * BC : (j + 1) * BC]
                        col = 128 * m
                        if which == "xb":
                            nc.tensor.matmul(
                                out,
                                wih0b[32 * idx : 32 * idx + KT, col : col + 128],
                                xb4[32 * idx : 32 * idx + KT, :],
                                start=True, stop=False,
                                tile_position=(32 * idx, 0),
                            )
                        elif which == "xa":
                            nc.tensor.matmul(out, wih0a[:, col : col + 128], xa[:],
                                             start=False, stop=(t == 0))
                        else:
                            for k in range(2):
                                nc.tensor.matmul(
                                    out,
                                    whh0[:, k * G + col : k * G + col + 128],
                                    h0[:, k * BC : (k + 1) * BC],
                                    start=False, stop=(k == 1),
                                )

                quad("xb")
                quad("xa")
                if t > 0:
                    quad("hh")
                for g in pair:
                    ps, sb = tiles[g]
                    # bias already in PSUM via the ones-row
                    if opts.get("l0_split_act"):
                        for j in range(2):
                            sl = slice(j * BC, (j + 1) * BC)
                            nc.scalar.activation(sb[:, sl], ps[:, sl], GATE_FUNCS[g])
                    else:
                        nc.scalar.activation(sb[:], ps[:], GATE_FUNCS[g])
            cell(t, "l0", gates, h0, c0)

        def l1_layer(t):
            """Layer 1 for step t; per gate-pair all hh matmuls precede ih
            matmuls to cover the h0[t] ACT/DVE tail with PE work."""
            gates = [None] * 4
            for g0 in (0, 2):
                tiles = {g: make_tiles(t, "l1", g) for g in (g0, g0 + 1)}
                for g in (g0, g0 + 1):
                    gates[g] = tiles[g][1]
                rhs_ih = [h0[:, 0:BC], h0[:, BC : 2 * BC]]
                if t > 0:
                    for g in (g0, g0 + 1):
                        for j in range(2):
                            col = 128 * (2 * g + j)
                            out = tiles[g][0][:, j * BC : (j + 1) * BC]
                            for k in range(2):
                                nc.tensor.matmul(
                                    out,
                                    whh1[:, k * G + col : k * G + col + 128],
                                    h1[:, k * BC : (k + 1) * BC],
                                    start=(k == 0), stop=False,
                                )
                for g in (g0, g0 + 1):
                    ps, sb = tiles[g]
                    for j in range(2):
                        m = 2 * g + j
                        col = 128 * m
                        out = ps[:, j * BC : (j + 1) * BC]
                        for k in range(2):
                            nc.tensor.matmul(
                                out,
                                wih1[:, k * G + col : k * G + col + 128],
                                rhs_ih[k],
                                start=(t == 0 and k == 0), stop=(k == 1),
                            )
                        nc.scalar.activation(sb[:, j * BC : (j + 1) * BC], out,
                                             GATE_FUNCS[g], bias=bias1[:, m : m + 1])
            cell(t, "l1", gates, h1, c1)

        def mlp_head(t):
            """Advantage head for step t; reads current h1 contents."""
            ps_a = psg.tile([128, 2 * BC], f32, tag="gates", name=f"ps_a1_{t}")
            relu = tpool.tile([128, 2 * BC], bf16, tag="relu", name=f"relu_{t}")
            for j in range(2):
                out = ps_a[:, j * BC : (j + 1) * BC]
                for k in range(2):
                    nc.tensor.matmul(
                        out,
                        wa1[:, k * H + 128 * j : k * H + 128 * j + 128],
                        h1[:, k * BC : (k + 1) * BC],
                        start=(k == 0), stop=(k == 1),
                    )
                nc.scalar.activation(relu[:, j * BC : (j + 1) * BC], out,
                                     AF.Relu, bias=ba1[:, j : j + 1])
            ps_o = pso.tile([1, BC], f32, tag="po", name=f"ps_o_{t}")
            for k in range(2):
                nc.tensor.matmul(ps_o[:], wa2[:, k : k + 1],
                                 relu[:, k * BC : (k + 1) * BC],
                                 start=(k == 0), stop=(k == 1))
            # b_a2 is added on the host; ACT drains PSUM (ACT has slack)
            osb = tpool.tile([1, BC], f32, tag="osb", name=f"osb_{t}")
            nc.scalar.copy(osb[:], ps_o[:])
            nc.sync.dma_start(o_d[t : t + 1, :], osb[:])

        for t in range(T_steps):
            xa = xpool.tile([128, BC], bf16, tag="xa", name=f"xa_{t}")
            nc.sync.dma_start(xa[:], xT_d[t, 0:128, :])
            xb4 = xpool.tile([96 + KT, BC], bf16, tag="xb", name=f"xb_{t}")
            for i in range(4):
                nc.sync.dma_start(xb4[32 * i : 32 * i + KT, :],
                                  xT_d[t, 128 : IN + 1, :])

            l0_layer(t, xa, xb4)
            if t > 0:
                mlp_head(t - 1)
            l1_layer(t)
        mlp_head(T_steps - 1)

    return _patch_bass(nc)


# --------------------------------------------------------------------------
# Host-side input prep / output assembly
# --------------------------------------------------------------------------
def _fold(wT: np.ndarray) -> np.ndarray:
    """(2K, M) -> (128, 2M): free halves are K-rows [0:128) / [128:256)."""
    k2, m = wT.shape
    assert k2 == 256
    return np.ascontiguousarray(
        wT.reshape(2, 128, m).transpose(1, 0, 2).reshape(128, 2 * m)
    )


def prepare_in_maps(inputs: dict) -> list[dict]:
    f32 = np.float32
    W_ih0 = np.asarray(inputs["W_ih0"], f32)
    W_hh0 = np.asarray(inputs["W_hh0"], f32)
    W_ih1 = np.asarray(inputs["W_ih1"], f32)
    W_hh1 = np.asarray(inputs["W_hh1"], f32)
    W_a1 = np.asarray(inputs["W_a1"], f32)
    W_a2 = np.asarray(inputs["W_a2"], f32)

    b0 = np.asarray(inputs["b_ih0"], f32) + np.asarray(inputs["b_hh0"], f32)
    # x-tail weight block: rows 128:140 of W_ih0.T plus the b0 bias row
    # (multiplied by the ones-row appended to x), replicated at partition
    # offsets 0/32/64/96 for tile_position-packed K=13 matmuls
    KT = IN + 1 - 128
    tail = np.concatenate([W_ih0.T[128:IN], b0[None, :]], axis=0)  # (13, G)
    wih0b = np.zeros((96 + KT, tail.shape[1]), f32)
    for i in range(4):
        wih0b[32 * i : 32 * i + KT] = tail

    shared = {
        "wih0": np.ascontiguousarray(W_ih0.T[0:128]).astype(BF16),
        "wih0b": wih0b.astype(BF16),
        "whh0": _fold(W_hh0.T).astype(BF16),
        "wih1": _fold(W_ih1.T).astype(BF16),
        "whh1": _fold(W_hh1.T).astype(BF16),
        "wa1": _fold(W_a1.T).astype(BF16),
        "wa2": _fold(W_a2.T).astype(BF16),
        "bias1": np.ascontiguousarray(
            (np.asarray(inputs["b_ih1"], f32) + np.asarray(inputs["b_hh1"], f32))
            .reshape(8, 128).T),
        "ba1": np.ascontiguousarray(np.asarray(inputs["b_a1"], f32).reshape(2, 128).T),
    }

    x = np.asarray(inputs["x"], f32)  # (B, T, IN)
    t_steps = x.shape[1]
    xT = x.transpose(1, 2, 0)  # (T, IN, B) view
    in_maps = []
    for c in range(N_CORES):
        xc = np.empty((t_steps, IN + 1, BC), BF16)
        xc[:, :IN, :] = xT[:, :, c * BC : (c + 1) * BC].astype(BF16)
        xc[:, IN, :] = np.ones((), BF16)
        in_maps.append({"xT": xc, **shared})
    return in_maps


def assemble_output(results: list[dict], b_a2_val: float) -> np.ndarray:
    out_tb = np.concatenate([r["o"] for r in results], axis=1)  # (T, B)
    out_tb = out_tb + np.float32(b_a2_val)
    t_steps = out_tb.shape[0]
    return np.ascontiguousarray(out_tb.reshape(B, t_steps))


_module_cache: dict = {}


def get_module(b_a2_val: float):
    key = round(float(b_a2_val), 12)
    if key not in _module_cache:
        _module_cache[key] = build_module(float(b_a2_val))
    return _module_cache[key]


def kernel(**inputs) -> np.ndarray:
    from concourse import bass_utils

    b_a2_val = float(np.asarray(inputs["b_a2"], np.float32).reshape(-1)[0])
    nc = get_module(b_a2_val)
    in_maps = prepare_in_maps(inputs)
    res = bass_utils.run_bass_kernel_spmd(nc, in_maps, core_ids=list(range(N_CORES)))
    return assemble_output(res.results, b_a2_val)


# revision 32
# speedup vs baseline: 1.0232x; 1.0220x over previous
"""Trainium2 Bass kernel for nn_DuelingDQN (2-layer LSTM + dueling-advantage MLP).

Strategy
--------
Data-parallel over batch: B=4096 is split as 512 per NeuronCore across 8 cores;
weights are replicated. On each core everything is kept in a transposed layout
(features on the SBUF partition dim, batch on the free dim), so the sequential
T=100 recurrence runs as a chain of bf16 matmuls (fp32 PSUM accumulation):

    gates.T (1024 x 512) = W.T-slices.T @ [x_t.T ; h.T]

256-row feature tensors (h, c, per-gate activations) are stored "folded" as
(128, 2*512) SBUF tiles — free-dim halves are feature rows [0:128) / [128:256) —
which halves the elementwise-op count. The input x is pre-transposed and cast
to bf16 on the host (numpy), so the device never transposes anything.

Per step: 64 LSTM matmuls + 6 MLP matmuls on PE; gate sigmoid/tanh on ACT
straight out of PSUM; cell updates on DVE (c stays fp32, h is written as bf16
for the next matmul). Layer-0's bias rides the K=13 x-tail matmul via a
ones-row appended to x, and the four tail matmuls of a gate quad issue at
tile_position row-groups 0/32/64/96 so they run concurrently in the PE array.
Layer-1 emits its recurrent matmuls for a gate pair before the input matmuls,
and the MLP head for step t is emitted in the middle of step t+1, so PE keeps
streaming across the h0/h1 ACT+DVE tails of the recurrence.

The walrus build in this container encodes at most ONE sync-wait per
instruction; Tile emits several. `_split_multiwaits` post-processes the BIR
JSON, hoisting extra waits onto injected same-engine EventSemaphore
instructions immediately before the owner (engine streams are in-order, so
this is semantically identical).
"""

import json
import sys
import types
from contextlib import ExitStack

import numpy as np

sys.path.insert(0, "/opt/trn_rl_repo")

import ml_dtypes  # noqa: E402

N_CORES = 8
B, T, IN, H = 4096, 100, 140, 256
BC = B // N_CORES  # 512 batch per core
G = 4 * H  # 1024 gate rows
BF16 = ml_dtypes.bfloat16


# --------------------------------------------------------------------------
# BIR post-processing: split multi-wait instructions (see module docstring)
# --------------------------------------------------------------------------
def _split_multiwaits(bir: dict) -> int:
    ctr = 0
    for f in bir["functions"]:
        for blk in f["blocks"]:
            new_insts = []
            for ins in blk["instructions"]:
                si = ins.get("sync_info")
                waits = (si or {}).get("on_wait") or []
                if len(waits) > 1:
                    for w in waits[:-1]:
                        ctr += 1
                        new_insts.append(
                            {
                                "debug": ins.get("debug", 0),
                                "engine": ins["engine"],
                                "ins": [],
                                "outs": [],
                                "name": f"antsplitw-{ctr}",
                                "opcode": "EventSemaphore",
                                "sync_info": {"on_update": [], "on_wait": [w]},
                            }
                        )
                    si["on_wait"] = [waits[-1]]
                new_insts.append(ins)
            blk["instructions"] = new_insts
    return ctr


def _patch_bass(nc):
    import concourse.mybir as mybir

    def to_json_bytes(self):
        j = json.loads(mybir.module_to_json_bytes(self.m))
        _split_multiwaits(j)
        return json.dumps(j).encode()

    nc.to_json_bytes = types.MethodType(to_json_bytes, nc)
    return nc


# --------------------------------------------------------------------------
# Module build
# --------------------------------------------------------------------------
def build_module(b_a2_val: float, T_steps: int = T, opts: dict | None = None):
    opts = opts or {}
    import concourse.bass as bass
    import concourse.tile as tile
    from concourse import mybir

    f32 = mybir.dt.float32
    bf16 = mybir.dt.bfloat16
    AF = mybir.ActivationFunctionType
    ALU = mybir.AluOpType

    nc = bass.Bass("TRN2", target_bir_lowering=False, debug=False)

    # x is extended with a constant ones-row (index IN) so the layer-0 bias
    # rides in the tail matmul (wih0 row IN = b0) — frees ACT from per-half
    # bias and lets layer-0 gates use single folded 1024-wide ACT ops.
    xT_d = nc.dram_tensor("xT", (T_steps, IN + 1, BC), bf16, kind="ExternalInput").ap()
    wih0_d = nc.dram_tensor("wih0", (128, G), bf16, kind="ExternalInput").ap()
    # x-tail weights (12 rows of W_ih0.T + b0 row), replicated at partition
    # offsets 0/32/64/96 (host-prepared)
    wih0b_d = nc.dram_tensor("wih0b", (96 + (IN + 1 - 128), G), bf16,
                             kind="ExternalInput").ap()
    whh0_d = nc.dram_tensor("whh0", (128, 2 * G), bf16, kind="ExternalInput").ap()
    wih1_d = nc.dram_tensor("wih1", (128, 2 * G), bf16, kind="ExternalInput").ap()
    whh1_d = nc.dram_tensor("whh1", (128, 2 * G), bf16, kind="ExternalInput").ap()
    wa1_d = nc.dram_tensor("wa1", (128, 2 * H), bf16, kind="ExternalInput").ap()
    wa2_d = nc.dram_tensor("wa2", (128, 2), bf16, kind="ExternalInput").ap()
    bias1_d = nc.dram_tensor("bias1", (128, 8), f32, kind="ExternalInput").ap()
    ba1_d = nc.dram_tensor("ba1", (128, 2), f32, kind="ExternalInput").ap()
    o_d = nc.dram_tensor("o", (T_steps, BC), f32, kind="ExternalOutput").ap()

    GATE_FUNCS = [AF.Sigmoid, AF.Sigmoid, AF.Tanh, AF.Sigmoid]  # i, f, g, o

    with tile.TileContext(nc) as tc, ExitStack() as ctx:
        persist = ctx.enter_context(tc.tile_pool(name="persist", bufs=1))
        xpool = ctx.enter_context(tc.tile_pool(name="xpool", bufs=4))
        gpool = ctx.enter_context(tc.tile_pool(name="gates_sb", bufs=opts.get("gbufs", 3)))
        tpool = ctx.enter_context(tc.tile_pool(name="tmp_sb", bufs=opts.get("tbufs", 3)))
        psg = ctx.enter_context(
            tc.tile_pool(name="ps_gates", bufs=opts.get("psbufs", 3), space="PSUM"))
        if opts.get("psbufs", 3) == 3:
            pso = ctx.enter_context(tc.tile_pool(name="ps_out", bufs=2, space="PSUM"))
        else:
            pso = psg  # mlp2 output shares the gates slots (frees 2 banks)

        def load(name, dram_ap, shape, dt):
            t = persist.tile(shape, dt, tag=name, name=name)
            nc.sync.dma_start(t[:], dram_ap)
            return t

        KT = IN + 1 - 128  # 13 tail rows (12 x rows + ones row)
        wih0a = load("wih0a", wih0_d[:], [128, G], bf16)
        # tail weights replicated at partition offsets 0/32/64/96 so four K=13
        # tail matmuls can run concurrently in distinct PE row-groups
        wih0b = load("wih0b", wih0b_d[:], [96 + KT, G], bf16)
        whh0 = load("whh0", whh0_d[:], [128, 2 * G], bf16)
        wih1 = load("wih1", wih1_d[:], [128, 2 * G], bf16)
        whh1 = load("whh1", whh1_d[:], [128, 2 * G], bf16)
        wa1 = load("wa1", wa1_d[:], [128, 2 * H], bf16)
        wa2 = load("wa2", wa2_d[:], [128, 2], bf16)
        bias1 = load("bias1", bias1_d[:], [128, 8], f32)
        ba1 = load("ba1", ba1_d[:], [128, 2], f32)

        h0 = persist.tile([128, 2 * BC], bf16, tag="h0", name="h0")
        h1 = persist.tile([128, 2 * BC], bf16, tag="h1", name="h1")
        cdt = bf16 if opts.get("c_bf16") else f32
        c0 = persist.tile([128, 2 * BC], cdt, tag="c0", name="c0")
        c1 = persist.tile([128, 2 * BC], cdt, tag="c1", name="c1")

        def make_tiles(t, lname, g):
            ps = psg.tile([128, 2 * BC], f32, tag="gates", name=f"ps_{lname}{g}_{t}")
            sb = gpool.tile([128, 2 * BC], bf16, tag=f"g{g}",
                            name=f"sb_{lname}{g}_{t}")
            return ps, sb

        def cell(t, lname, gates, h, c):
            gi, gf, gg, go = gates
            if t > 0:
                # c*f first: it only waits on the f-gate ACT (ready mid-layer)
                nc.vector.tensor_mul(c[:], c[:], gf[:])
                t1 = tpool.tile([128, 2 * BC], bf16, tag="t1", name=f"t1_{lname}_{t}")
                nc.vector.tensor_mul(t1[:], gi[:], gg[:])
                nc.vector.tensor_add(c[:], c[:], t1[:])
            else:
                nc.vector.tensor_mul(c[:], gi[:], gg[:])
            # halves: the consumer's first K-tile matmul only needs h[:, 0:BC]
            tc_t = tpool.tile([128, 2 * BC], bf16, tag="tanhc", name=f"tc_{lname}_{t}")
            for j in range(2):
                sl = slice(j * BC, (j + 1) * BC)
                nc.scalar.activation(tc_t[:, sl], c[:, sl], AF.Tanh)
                nc.vector.tensor_mul(h[:, sl], go[:, sl], tc_t[:, sl])

        def l0_layer(t, xa, xb4, mid_hook=None):
            """Layer 0 for step t: bias rides in the K=13 tail matmul whose
            four per-quad instances issue back-to-back at distinct
            tile_positions (row-groups 0/32/64/96) and run concurrently."""
            gates = [None] * 4
            for q in (0, 1):
                pair = (2 * q, 2 * q + 1)
                tiles = {}
                for g in pair:
                    tiles[g] = make_tiles(t, "l0", g)
                    gates[g] = tiles[g][1]

                def quad(which):
                    for idx in range(4):
                        m = 4 * q + idx
                        g, j = divmod(m, 2)
                        out = tiles[g][0][:, j * BC : (j + 1) * BC]
                        col = 128 * m
                        if which == "xb":
                            nc.tensor.matmul(
                                out,
                                wih0b[32 * idx : 32 * idx + KT, col : col + 128],
                                xb4[32 * idx : 32 * idx + KT, :],
                                start=True, stop=False,
                                tile_position=(32 * idx, 0),
                            )
                        elif which == "xa":
                            nc.tensor.matmul(out, wih0a[:, col : col + 128], xa[:],
                                             start=False, stop=(t == 0))
                        else:
                            for k in range(2):
                                nc.tensor.matmul(
                                    out,
                                    whh0[:, k * G + col : k * G + col + 128],
                                    h0[:, k * BC : (k + 1) * BC],
                                    start=False, stop=(k == 1),
                                )

                quad("xb")
                quad("xa")
                if t > 0:
                    quad("hh")
                for g in pair:
                    ps, sb = tiles[g]
                    # bias already in PSUM via the ones-row
                    if opts.get("l0_split_act"):
                        for j in range(2):
                            sl = slice(j * BC, (j + 1) * BC)
                            nc.scalar.activation(sb[:, sl], ps[:, sl], GATE_FUNCS[g])
                    else:
                        nc.scalar.activation(sb[:], ps[:], GATE_FUNCS[g])
            if mid_hook is not None:
                # MLP[t-1] ACT/PE work lands between the gate ACTs and the
                # cell's tanh, so ACT's in-order queue isn't head-of-line
                # blocked waiting for the DVE c-chain
                mid_hook()
            cell(t, "l0", gates, h0, c0)

        def l1_layer(t):
            """Layer 1 for step t; per gate-pair all hh matmuls precede ih
            matmuls to cover the h0[t] ACT/DVE tail with PE work."""
            gates = [None] * 4
            for g0 in (0, 2):
                tiles = {g: make_tiles(t, "l1", g) for g in (g0, g0 + 1)}
                for g in (g0, g0 + 1):
                    gates[g] = tiles[g][1]
                rhs_ih = [h0[:, 0:BC], h0[:, BC : 2 * BC]]
                if t > 0:
                    for g in (g0, g0 + 1):
                        for j in range(2):
                            col = 128 * (2 * g + j)
                            out = tiles[g][0][:, j * BC : (j + 1) * BC]
                            for k in range(2):
                                nc.tensor.matmul(
                                    out,
                                    whh1[:, k * G + col : k * G + col + 128],
                                    h1[:, k * BC : (k + 1) * BC],
                                    start=(k == 0), stop=False,
                                )
                for g in (g0, g0 + 1):
                    ps, sb = tiles[g]
                    for j in range(2):
                        m = 2 * g + j
                        col = 128 * m
                        out = ps[:, j * BC : (j + 1) * BC]
                        for k in range(2):
                            nc.tensor.matmul(
                                out,
                                wih1[:, k * G + col : k * G + col + 128],
                                rhs_ih[k],
                                start=(t == 0 and k == 0), stop=(k == 1),
                            )
                        nc.scalar.activation(sb[:, j * BC : (j + 1) * BC], out,
                                             GATE_FUNCS[g], bias=bias1[:, m : m + 1])
            cell(t, "l1", gates, h1, c1)

        def mlp_head(t):
            """Advantage head for step t; reads current h1 contents."""
            ps_a = psg.tile([128, 2 * BC], f32, tag="gates", name=f"ps_a1_{t}")
            relu = tpool.tile([128, 2 * BC], bf16, tag="relu", name=f"relu_{t}")
            for j in range(2):
                out = ps_a[:, j * BC : (j + 1) * BC]
                for k in range(2):
                    nc.tensor.matmul(
                        out,
                        wa1[:, k * H + 128 * j : k * H + 128 * j + 128],
                        h1[:, k * BC : (k + 1) * BC],
                        start=(k == 0), stop=(k == 1),
                    )
                nc.scalar.activation(relu[:, j * BC : (j + 1) * BC], out,
                                     AF.Relu, bias=ba1[:, j : j + 1])
            po_tag = "gates" if pso is psg else "po"
            ps_o = pso.tile([1, BC], f32, tag=po_tag, name=f"ps_o_{t}")
            for k in range(2):
                nc.tensor.matmul(ps_o[:], wa2[:, k : k + 1],
                                 relu[:, k * BC : (k + 1) * BC],
                                 start=(k == 0), stop=(k == 1))
            # b_a2 is added on the host; ACT drains PSUM (ACT has slack)
            osb = tpool.tile([1, BC], f32, tag="osb", name=f"osb_{t}")
            nc.scalar.copy(osb[:], ps_o[:])
            nc.sync.dma_start(o_d[t : t + 1, :], osb[:])

        for t in range(T_steps):
            xa = xpool.tile([128, BC], bf16, tag="xa", name=f"xa_{t}")
            nc.sync.dma_start(xa[:], xT_d[t, 0:128, :])
            xb4 = xpool.tile([96 + KT, BC], bf16, tag="xb", name=f"xb_{t}")
            for i in range(4):
                nc.sync.dma_start(xb4[32 * i : 32 * i + KT, :],
                                  xT_d[t, 128 : IN + 1, :])

            l0_layer(t, xa, xb4,
                     mid_hook=(lambda tt=t: mlp_head(tt - 1)) if t > 0 else None)
            l1_layer(t)
        mlp_head(T_steps - 1)

    return _patch_bass(nc)


# --------------------------------------------------------------------------
# Host-side input prep / output assembly
# --------------------------------------------------------------------------
def _fold(wT: np.ndarray) -> np.ndarray:
    """(2K, M) -> (128, 2M): free halves are K-rows [0:128) / [128:256)."""
    k2, m = wT.shape
    assert k2 == 256
    return np.ascontiguousarray(
        wT.reshape(2, 128, m).transpose(1, 0, 2).reshape(128, 2 * m)
    )


def prepare_in_maps(inputs: dict) -> list[dict]:
    f32 = np.float32
    W_ih0 = np.asarray(inputs["W_ih0"], f32)
    W_hh0 = np.asarray(inputs["W_hh0"], f32)
    W_ih1 = np.asarray(inputs["W_ih1"], f32)
    W_hh1 = np.asarray(inputs["W_hh1"], f32)
    W_a1 = np.asarray(inputs["W_a1"], f32)
    W_a2 = np.asarray(inputs["W_a2"], f32)

    b0 = np.asarray(inputs["b_ih0"], f32) + np.asarray(inputs["b_hh0"], f32)
    # x-tail weight block: rows 128:140 of W_ih0.T plus the b0 bias row
    # (multiplied by the ones-row appended to x), replicated at partition
    # offsets 0/32/64/96 for tile_position-packed K=13 matmuls
    KT = IN + 1 - 128
    tail = np.concatenate([W_ih0.T[128:IN], b0[None, :]], axis=0)  # (13, G)
    wih0b = np.zeros((96 + KT, tail.shape[1]), f32)
    for i in range(4):
        wih0b[32 * i : 32 * i + KT] = tail

    shared = {
        "wih0": np.ascontiguousarray(W_ih0.T[0:128]).astype(BF16),
        "wih0b": wih0b.astype(BF16),
        "whh0": _fold(W_hh0.T).astype(BF16),
        "wih1": _fold(W_ih1.T).astype(BF16),
        "whh1": _fold(W_hh1.T).astype(BF16),
        "wa1": _fold(W_a1.T).astype(BF16),
        "wa2": _fold(W_a2.T).astype(BF16),
        "bias1": np.ascontiguousarray(
            (np.asarray(inputs["b_ih1"], f32) + np.asarray(inputs["b_hh1"], f32))
            .reshape(8, 128).T),
        "ba1": np.ascontiguousarray(np.asarray(inputs["b_a1"], f32).reshape(2, 128).T),
    }

    x = np.asarray(inputs["x"], f32)  # (B, T, IN)
    t_steps = x.shape[1]
    xT = x.transpose(1, 2, 0)  # (T, IN, B) view
    in_maps = []
    for c in range(N_CORES):
        xc = np.empty((t_steps, IN + 1, BC), BF16)
        xc[:, :IN, :] = xT[:, :, c * BC : (c + 1) * BC].astype(BF16)
        xc[:, IN, :] = np.ones((), BF16)
        in_maps.append({"xT": xc, **shared})
    return in_maps


def assemble_output(results: list[dict], b_a2_val: float) -> np.ndarray:
    out_tb = np.concatenate([r["o"] for r in results], axis=1)  # (T, B)
    out_tb = out_tb + np.float32(b_a2_val)
    t_steps = out_tb.shape[0]
    return np.ascontiguousarray(out_tb.reshape(B, t_steps))


_module_cache: dict = {}


def get_module(b_a2_val: float):
    key = round(float(b_a2_val), 12)
    if key not in _module_cache:
        _module_cache[key] = build_module(float(b_a2_val))
    return _module_cache[key]


def kernel(**inputs) -> np.ndarray:
    from concourse import bass_utils

    b_a2_val = float(np.asarray(inputs["b_a2"], np.float32).reshape(-1)[0])
    nc = get_module(b_a2_val)
    in_maps = prepare_in_maps(inputs)
    res = bass_utils.run_bass_kernel_spmd(nc, in_maps, core_ids=list(range(N_CORES)))
    return assemble_output(res.results, b_a2_val)
